# revision 39
# baseline (speedup 1.0000x reference)
"""BotRGCN (2x RGCNConv + MLPs) on 8 Trainium2 NeuronCores.

Strategy (v2): shard by destination node (each core owns 12500 dst nodes).
Edge slots are ordered (superblock, relation, src-window, dst-block) so that:
  - per-(sb, r) PSUM mean-chains accumulate across all 4 source windows and
    drain ONCE (copy, not read-modify-write adds),
  - the per-relation W_rel / W_root transform accumulates across relations in
    a PSUM agg-chain (no SBUF read-modify-write adds on DVE),
  - sel one-hot tiles are generated per gather call (2 DVE ops for ~8
    batches via broadcast APs) instead of per batch,
  - the node table is laid out quarter-major so each layer's AllGather is 4
    chunked collectives that overlap the tail superblocks' compute,
  - gather calls stay at <=1024 edges (8 batches) - measured optimum for
    SWDGE gen/transfer/compute pipelining.
dma_scatter_add is NOT used: its HBM read-modify-write races on duplicate
indices (verified on HW), which segment-sum requires.
"""
import os as _os

import numpy as np

import concourse.bacc as bacc
import concourse.mybir as mybir
import concourse.tile as tile
from concourse.bass_utils import run_bass_kernel_spmd

# ---------------- problem constants (hardcoded per the task contract) ----
N, E, R, D = 100000, 1600000, 5, 128
C = 8                     # cores
NSLAB = 12500             # real nodes owned per core
NBLK = 104                # 128-dst blocks per core (padded slab)
SLAB = NBLK * 128         # 13312 padded slab rows per core
NRBLK = 98                # blocks with any real dst (12500/128 -> 97.6)
QROWS = 3328              # quarter-slab rows (SLAB/4)
NQ = 4
W = 4                     # source windows == source quarters
WIN = C * QROWS           # 26624 table rows per window (< 32768 for int16)
SB = 8                    # blocks per superblock
NSB = 13                  # superblocks (sb12 holds real blocks 96..97)
NPAD = C * SLAB           # 106496 table rows
CALLB = int(_os.environ.get("GNN_CALLB", "16"))  # max batches per gather call
F16 = mybir.dt.float16
F32 = mybir.dt.float32
I16 = mybir.dt.int16

_AluOp = mybir.AluOpType
_Act = mybir.ActivationFunctionType


# ---------------- walrus workaround --------------------------------------
def _split_sync_waits(nc, maxw=1):
    """walrus build here rejects >1 sync wait per instruction; hoist excess
    waits onto same-engine InstDrain instructions inserted just before."""
    n_split = 0
    for fn in nc.m.functions:
        for bb in fn.blocks:
            new_insts = []
            for inst in bb.instructions:
                si = inst.sync_info
                if si is not None and si.on_wait and len(si.on_wait) > maxw:
                    waits = list(si.on_wait)
                    excess, keep = waits[:-maxw], waits[-maxw:]
                    for i in range(0, len(excess), maxw):
                        d = mybir.InstNoOp(name=f"waitsplit_{n_split}", ins=[], outs=[])
                        n_split += 1
                        d.engine = inst.engine
                        d.sync_info = mybir.SyncInfo(
                            on_wait=excess[i:i + maxw], on_update=[])
                        nc.register_instruction(d)
                        new_insts.append(d)
                    si.on_wait = keep
                new_insts.append(inst)
            bb.instructions[:] = new_insts
    return n_split


# ---------------- host-side prep ------------------------------------------
def _wrap_idx(a):
    """[C, TOT] int16 -> [C, 128, TOT//16]: slot i at (i%16, i//16), x8 replicated."""
    Cc, TOT = a.shape
    t = a.reshape(Cc, TOT // 16, 16).transpose(0, 2, 1)
    return np.ascontiguousarray(np.tile(t, (1, 8, 1)))


def _host_prep(edge_index, edge_type):
    src = np.asarray(edge_index[0], dtype=np.int64)
    dst = np.asarray(edge_index[1], dtype=np.int64)
    et = np.asarray(edge_type, dtype=np.int64)

    owner = dst // NSLAB
    dloc = dst - owner * NSLAB
    blk = dloc >> 7
    sb = blk >> 3
    srow = (src // NSLAB) * SLAB + (src % NSLAB)
    q = srow // WIN                         # source window
    widx = (srow - q * WIN).astype(np.int16)

    cnt_full = np.bincount(et * N + dst, minlength=R * N)
    dl7 = (dloc & 127).astype(np.float32)
    inv = (1.0 / np.maximum(cnt_full[et * N + dst], 1)).astype(np.float32)

    # group key order: (sb, r, w, blk): gather calls are w-pure within a
    # (sb, r) segment; fine per-(sb, r) gather->compute granularity measured
    # faster than coarser (sb, w)-spanning calls despite 2x SWDGE overhead
    g = ((sb * R + et) * W + q) * NBLK + blk
    G = NSB * R * W * NBLK
    counts = np.zeros((C, G), np.int64)
    np.add.at(counts, (owner, g), 1)
    B = -(-counts.max(axis=0) // 128)       # [G] batches (0 for empty groups)
    base_b = np.zeros(G, np.int64)
    base_b[1:] = np.cumsum(B)[:-1]          # batch index base per group
    NB = int(B.sum())
    TOT = NB * 128

    # per-core slot placement
    xidx = np.zeros((C, TOT), np.int16)
    mdl = np.zeros((C, TOT), np.float32)
    minv = np.zeros((C, TOT), np.float32)
    key = owner * G + g
    order = np.argsort(key, kind="stable")
    ks = key[order]
    grp_start = np.r_[0, np.flatnonzero(np.diff(ks)) + 1]
    grp_len = np.diff(np.r_[grp_start, E])
    ranks = np.arange(E) - np.repeat(grp_start, grp_len)
    pos = base_b[g[order]] * 128 + ranks
    xidx[owner[order], pos] = widx[order]
    mdl[owner[order], pos] = dl7[order]
    minv[owner[order], pos] = inv[order]

    # schedule: per sb -> per r -> calls (w-pure, <=CALLB batches) + matmuls
    Bv = B.reshape(NSB, R, W, NBLK)
    base_v = base_b.reshape(NSB, R, W, NBLK)
    sched = []
    qno = 0
    for s in range(NSB):
        width = (min(NRBLK, (s + 1) * SB) - s * SB) * 128
        rsegs = []
        for r in range(R):
            # batch records in (w, blk) order == slot order
            recs = []          # (w, blk)
            for w in range(W):
                for b in range(SB):
                    nbl = int(Bv[s, r, w, s * SB + b])
                    recs += [(w, b)] * nbl
            nb = len(recs)
            if nb == 0:
                rsegs.append(None)
                continue
            g0 = int(base_v[s, r, 0, s * SB])  # first batch of this (sb, r)
            # calls: split at w boundaries, cap CALLB batches (slot order)
            calls = []
            i = 0
            while i < nb:
                w = recs[i][0]
                j = i
                while j < nb and recs[j][0] == w and j - i < CALLB:
                    j += 1
                calls.append({"off": i, "nb": j - i, "w": w, "q": qno % 4})
                qno += 1
                i = j
            # seg-position -> (call idx, idx within call)
            pos2call = []
            for ci, call in enumerate(calls):
                pos2call += [(ci, k) for k in range(call["nb"])]
            # seg-position of first batch of each (w, blk) group (slot order)
            gpos = {}
            p = 0
            for w in range(W):
                for b in range(SB):
                    nbl = int(Bv[s, r, w, s * SB + b])
                    if nbl:
                        gpos[(w, b)] = p
                        p += nbl
            # matmul order: (blk, w, k) so each blk's psum chain is a
            # CONSECUTIVE run of matmuls (interleaved psum accumulation
            # chains are broken on HW - verified)
            mms = []
            for b in range(SB):
                wbs = [(w, int(Bv[s, r, w, s * SB + b])) for w in range(W)
                       if Bv[s, r, w, s * SB + b]]
                tot = sum(n for _, n in wbs)
                k = 0
                for w, nbl in wbs:
                    p0 = gpos[(w, b)]
                    for kk in range(nbl):
                        ci, ii = pos2call[p0 + kk]
                        mms.append({
                            "call": ci, "i": ii, "b": g0 + p0 + kk, "col": b,
                            "start": k == 0, "stop": k == tot - 1,
                        })
                        k += 1
            rsegs.append({"g0": g0, "nb": nb, "calls": calls, "mms": mms})
        sched.append({"width": width, "rsegs": rsegs})

    meta_dl = np.ascontiguousarray(
        mdl.reshape(C, NB, 128).transpose(0, 2, 1))   # [C, 128, NB]
    meta_inv = np.ascontiguousarray(
        minv.reshape(C, NB, 128).transpose(0, 2, 1))
    return xidx, (meta_dl, meta_inv), sched, NB, TOT


def _slabify_featT(num_prop, cat_prop):
    feat = np.concatenate([num_prop, cat_prop], axis=1)          # [N, 17]
    out = np.zeros((C, SLAB, 17), np.float16)
    out[:, :NSLAB] = feat.reshape(C, NSLAB, 17).astype(np.float16)
    featT = out.transpose(0, 2, 1)                               # [C, 17, SLAB]
    ones = np.ones((C, 1, SLAB), np.float16)
    return np.ascontiguousarray(np.concatenate([featT, ones], axis=1))


# ---------------- device program ------------------------------------------
def _build(sched, NB, TOT, skip=()):
    nc = bacc.Bacc("TRN2", target_bir_lowering=False, debug=False,
                   num_devices=C, num_swdge_queues=4)

    # inputs
    featT = nc.dram_tensor("featT", [18, SLAB], F16, kind="ExternalInput")
    xidx_d = nc.dram_tensor("xidx", [128, TOT // 16], I16, kind="ExternalInput")
    mdl_d = nc.dram_tensor("meta_dl", [128, NB], F32, kind="ExternalInput")
    minv_d = nc.dram_tensor("meta_inv", [128, NB], F32, kind="ExternalInput")
    iota_d = nc.dram_tensor("iota128", [128, 128], F16, kind="ExternalInput")
    wnc_d = nc.dram_tensor("wnc", [18, 128], F16, kind="ExternalInput")
    win_d = nc.dram_tensor("win", [128, 128], F16, kind="ExternalInput")
    wrel_d = nc.dram_tensor("wrel", [R * 128, 128], F16, kind="ExternalInput")
    wroot_d = nc.dram_tensor("wroot", [128, 128], F16, kind="ExternalInput")
    wo1_d = nc.dram_tensor("wo1", [128, 128], F16, kind="ExternalInput")
    wo2_d = nc.dram_tensor("wo2", [128, 2], F16, kind="ExternalInput")
    bias_d = nc.dram_tensor("biases", [128, 4], F32, kind="ExternalInput")
    ident_d = nc.dram_tensor("ident", [128, 128], F16, kind="ExternalInput")
    out_d = nc.dram_tensor("out", [2, SLAB], F32, kind="ExternalOutput")

    with tile.TileContext(nc) as tc:
        with (
            tc.tile_pool(name="const", bufs=1) as constp,
            tc.tile_pool(name="slabs", bufs=1) as slabp,
            tc.tile_pool(name="dram", bufs=1, space="DRAM") as dramp,
        ):
            def load_const(name, dram, shape):
                t = constp.tile(shape, F16, name=name)
                nc.sync.dma_start(t[:], dram[:])
                return t

            wnc = load_const("wnc", wnc_d, [18, 128])
            win = load_const("win", win_d, [128, 128])
            wrel = []
            for r in range(R):
                t = constp.tile([128, 128], F16, name=f"wrel{r}")
                nc.sync.dma_start(t[:], wrel_d[r * 128:(r + 1) * 128, :])
                wrel.append(t)
            wroot = load_const("wroot", wroot_d, [128, 128])
            wo1 = load_const("wo1", wo1_d, [128, 128])
            wo2 = load_const("wo2", wo2_d, [128, 2])
            ident = load_const("ident", ident_d, [128, 128])
            iota128 = load_const("iota128", iota_d, [128, 128])
            biases = constp.tile([128, 4], F32, name="biases")
            nc.sync.dma_start(biases[:], bias_d[:])
            meta_dl = constp.tile([128, NB], F32, name="meta_dl")
            nc.sync.dma_start(meta_dl[:], mdl_d[:])
            meta_inv = constp.tile([128, NB], F32, name="meta_inv")
            nc.sync.dma_start(meta_inv[:], minv_d[:])
            b_in = biases[:, 0:1]
            b_rgcn = biases[:, 1:2]
            b_o1 = biases[:, 2:3]
            b_o2 = biases[0:2, 3:4]

            # resident slabs (feature-major f16)
            xT_A = slabp.tile([128, SLAB], F16, name="xT_A")   # x0, later x2
            xT_B = slabp.tile([128, SLAB], F16, name="xT_B")   # x1

            # DRAM: per-layer slab + allgather table
            slab_l = [dramp.tile([SLAB, D], F16, name=f"slab{l}") for l in range(2)]
            tab_l = [dramp.tile([NPAD, D], F16, name=f"tab{l}", addr_space="Shared")
                     for l in range(2)]

            # ---- emit quarter qq of src_slab into slab of layer l; the
            # full-slab AllGather fires with the last quarter ---------------
            def emit_quarter(src_slab, l, qq):
                nblk = min(NRBLK, (qq + 1) * 26) - qq * 26
                with (
                    tc.tile_pool(name=f"tr{l}{qq}", bufs=1) as trp,
                    tc.tile_pool(name=f"trps{l}{qq}", bufs=2, space="PSUM") as trps,
                ):
                    stag = trp.tile([128, QROWS], F16, tag="stag")
                    for b in range(nblk):
                        bs = slice((qq * 26 + b) * 128, (qq * 26 + b + 1) * 128)
                        tp = trps.tile([128, 128], F16, tag="tp")
                        nc.tensor.transpose(tp[:], src_slab[:, bs], ident[:])
                        nc.scalar.copy(stag[:, b * 128:(b + 1) * 128], tp[:])
                    dst = slab_l[l][qq * QROWS:(qq + 1) * QROWS, :] \
                        .rearrange("(b p) f -> p b f", p=128)
                    nc.sync.dma_start(dst, stag[:].rearrange("p (b f) -> p b f", f=128))
                if qq == NQ - 1:
                    nc.gpsimd.collective_compute(
                        "AllGather", _AluOp.bypass,
                        ins=[slab_l[l].opt()], outs=[tab_l[l].opt()],
                        replica_groups=[list(range(C))])

            # ---------------- phase 0: node MLP -> x0 ---------------------
            with (
                tc.tile_pool(name="p0", bufs=3) as p0,
                tc.tile_pool(name="ps0", bufs=2, space="PSUM") as ps0,
            ):
                for qq in range(NQ):
                    ft = p0.tile([18, QROWS], F16, tag="ft")
                    nc.sync.dma_start(ft[:], featT[:, qq * QROWS:(qq + 1) * QROWS])
                    for t in range(7):
                        c0 = t * 512
                        c1 = min(c0 + 512, QROWS)
                        cs = slice(qq * QROWS + c0, qq * QROWS + c1)
                        pa = ps0.tile([128, 512], F32, tag="pa")
                        nc.tensor.matmul(pa[:, :c1 - c0], wnc[:], ft[:, c0:c1],
                                         start=True, stop=True)
                        xnc = p0.tile([128, 512], F16, tag="xnc")
                        nc.scalar.activation(xnc[:, :c1 - c0], pa[:, :c1 - c0],
                                             _Act.Lrelu, alpha=0.01)
                        pb = ps0.tile([128, 512], F32, tag="pb")
                        nc.tensor.matmul(pb[:, :c1 - c0], win[:], xnc[:, :c1 - c0],
                                         start=True, stop=True)
                        nc.scalar.activation(xT_A[:, cs], pb[:, :c1 - c0],
                                             _Act.Lrelu, bias=b_in, alpha=0.01)
                    if "agx" not in skip:
                        emit_quarter(xT_A, 0, qq)

            # ---------------- RGCN layer ----------------------------------
            def rgcn_layer(l, xT_prev, xT_next, emit_to):
                tab = tab_l[l]
                with (
                    tc.tile_pool(name=f"gidx{l}", bufs=3) as gip,
                    tc.tile_pool(name=f"gdat{l}", bufs=int(_os.environ.get("GNN_GBUFS", "8"))) as gdp,
                    tc.tile_pool(name=f"selp{l}", bufs=8) as selp,
                    tc.tile_pool(name=f"meanp{l}", bufs=6) as meanp,
                    tc.tile_pool(name=f"mps{l}", bufs=2, space="PSUM") as mps,
                    tc.tile_pool(name=f"aps{l}", bufs=1, space="PSUM") as aps,
                ):
                    do_comp = "compute" not in skip
                    for s, srec in enumerate(sched):
                        width = srec["width"]
                        means = []
                        for r in range(R):
                            seg = srec["rsegs"][r]
                            if seg is None:
                                means.append(None)
                                continue
                            g0, nb = seg["g0"], seg["nb"]
                            xi = gip.tile([128, nb * 8], I16, tag="xi")
                            nc.sync.dma_start(
                                xi[:], xidx_d[:, g0 * 8:(g0 + nb) * 8])
                            xgs = []
                            for call in seg["calls"]:
                                off, cnb, w = call["off"], call["nb"], call["w"]
                                ns = cnb * 128
                                xg = gdp.tile([128, CALLB, 128], F16, tag="xg")
                                nc.gpsimd.dma_gather(
                                    xg[:, :cnb, :],
                                    tab[w * WIN:(w + 1) * WIN, :],
                                    xi[:, off * 8:(off + cnb) * 8],
                                    ns, ns, D,
                                    single_packet=False, queue_num=call["q"])
                                xgs.append(xg)
                            if not do_comp:
                                means.append(None)
                                continue
                            mp = mps.tile([128, 1024], F32, tag="mp")
                            for m in seg["mms"]:
                                b0, col = m["b"], m["col"]
                                sel = selp.tile([128, 128], F16, tag="sel")
                                nc.vector.tensor_scalar(
                                    sel[:], iota128[:],
                                    meta_dl[:, b0:b0 + 1],
                                    meta_inv[:, b0:b0 + 1],
                                    op0=_AluOp.is_equal, op1=_AluOp.mult)
                                nc.tensor.matmul(
                                    mp[:, col * 128:(col + 1) * 128],
                                    xgs[m["call"]][:, m["i"], :], sel[:],
                                    start=m["start"], stop=m["stop"])
                            mean_r = meanp.tile([128, 1024], F16, tag="mean")
                            h = width // 2
                            nc.vector.tensor_copy(mean_r[:, :h], mp[:, :h])
                            nc.scalar.copy(mean_r[:, h:width], mp[:, h:width])
                            means.append(mean_r)
                        # transform: agg = sum_r W_r^T mean_r + W_root^T x_prev
                        agg = aps.tile([128, 1024], F32, tag="agg")
                        for j0 in range(0, width, 512):
                            j1 = min(j0 + 512, width)
                            first = True
                            for r in range(R):
                                if means[r] is None:
                                    continue
                                nc.tensor.matmul(
                                    agg[:, j0:j1], wrel[r][:], means[r][:, j0:j1],
                                    start=first, stop=False)
                                first = False
                            nc.tensor.matmul(
                                agg[:, j0:j1], wroot[:],
                                xT_prev[:, s * 1024 + j0:s * 1024 + j1],
                                start=first, stop=True)
                        nc.scalar.activation(
                            xT_next[:, s * 1024:s * 1024 + width],
                            agg[:, :width], _Act.Identity, bias=b_rgcn)
                        if emit_to is not None and s in (3, 6, 9, 12):
                            if "agx" not in skip:
                                emit_quarter(xT_next, emit_to, s // 3 - 1)

            rgcn_layer(0, xT_A, xT_B, emit_to=1)
            rgcn_layer(1, xT_B, xT_A, emit_to=None)

            # ---------------- final MLP ------------------------------------
            with (
                tc.tile_pool(name="pf", bufs=3) as pf,
                tc.tile_pool(name="psf", bufs=2, space="PSUM") as psf,
            ):
                for t in range(SLAB // 512):
                    cs = slice(t * 512, (t + 1) * 512)
                    pa = psf.tile([128, 512], F32, tag="fa")
                    nc.tensor.matmul(pa[:], wo1[:], xT_A[:, cs], start=True, stop=True)
                    o1 = pf.tile([128, 512], F16, tag="fo1")
                    nc.scalar.activation(o1[:], pa[:], _Act.Lrelu,
                                         bias=b_o1, alpha=0.01)
                    pb = psf.tile([2, 512], F32, tag="fb")
                    nc.tensor.matmul(pb[:], wo2[:], o1[:], start=True, stop=True)
                    ot = pf.tile([2, 512], F32, tag="fot")
                    nc.scalar.activation(ot[:], pb[:], _Act.Identity, bias=b_o2)
                    nc.sync.dma_start(out_d[:, cs], ot[:])

    nc.compile()
    _split_sync_waits(nc)
    return nc


_CACHE = {}
_RUNNER = None  # test harness hook: set by bench.py to reuse one compile


def _prepare(inputs, skip=()):
    num_prop = np.asarray(inputs["num_prop"], np.float32)
    cat_prop = np.asarray(inputs["cat_prop"], np.float32)
    edge_index = np.asarray(inputs["edge_index"])
    edge_type = np.asarray(inputs["edge_type"])

    xidx, (meta_dl, meta_inv), sched, NB, TOT = _host_prep(edge_index, edge_type)
    nc = _build(sched, NB, TOT, skip=skip)

    featT = _slabify_featT(num_prop, cat_prop)                   # [C, 18, SLAB]

    wnp = np.asarray(inputs["W_np"], np.float32)
    wcp = np.asarray(inputs["W_cp"], np.float32)
    wnc = np.zeros((18, 128), np.float16)
    wnc[0:6, 0:64] = wnp
    wnc[6:17, 64:128] = wcp
    wnc[17, 0:64] = np.asarray(inputs["b_np"], np.float32)
    wnc[17, 64:128] = np.asarray(inputs["b_cp"], np.float32)

    biases = np.zeros((128, 4), np.float32)
    biases[:, 0] = np.asarray(inputs["b_in"], np.float32)
    biases[:, 1] = np.asarray(inputs["b_rgcn"], np.float32)
    biases[:, 2] = np.asarray(inputs["b_o1"], np.float32)
    biases[0:2, 3] = np.asarray(inputs["b_o2"], np.float32)

    common = {
        "iota128": np.tile(np.arange(128, dtype=np.float16), (128, 1)),
        "wnc": wnc,
        "win": np.asarray(inputs["W_in"], np.float16),
        "wrel": np.asarray(inputs["W_rel"], np.float16).reshape(R * 128, 128),
        "wroot": np.asarray(inputs["W_root"], np.float16),
        "wo1": np.asarray(inputs["W_o1"], np.float16),
        "wo2": np.asarray(inputs["W_o2"], np.float16),
        "biases": biases,
        "ident": np.eye(128, dtype=np.float16),
    }
    xw = _wrap_idx(xidx)
    in_maps = []
    for c in range(C):
        m = dict(common)
        m["featT"] = np.ascontiguousarray(featT[c])
        m["xidx"] = xw[c]
        m["meta_dl"] = meta_dl[c]
        m["meta_inv"] = meta_inv[c]
        in_maps.append(m)
    return nc, in_maps


def kernel(**inputs) -> np.ndarray:
    nc, in_maps = _prepare(inputs)
    _CACHE["nc"] = nc
    _CACHE["in_maps"] = in_maps
    runner = _RUNNER if _RUNNER is not None else run_bass_kernel_spmd
    res = runner(nc, in_maps, list(range(C)))
    out = np.concatenate(
        [res.results[c]["out"][:, :NSLAB].T for c in range(C)], axis=0)
    return out.astype(np.float32)
